# revision 22
# baseline (speedup 1.0000x reference)
"""Trainium2 Bass kernel for nn_CF_68169720922624 (segment_reduce CF predictor).

Computation (see reference):
    ub[u]   = masked mean of rating_mtx[u, :] over nonzero entries
    score[b]= sum_u  S[user[b], u] * (R[u, item[b]] - ub[u])
    out[b]  = sigmoid(score[b] + user_bias[user[b]] + item_bias[item[b]] + gb) * 5

Sharding: the contraction dim (users u) is split across 8 cores (1024 each).
Core k receives:
  r8  [8064, 2048] fp8e4 : phase-A stream; DoubleRow-interleaved transposed
                           ratings: r8[128t+p, 1024i+j] = R8[u_lo+j, 256t+128i+p]
  rt  [16128, 1152] fp16 : RT[i, j] = R[u_lo+j, i]; col 1024 = 1.0,
                           col 1025 = item_bias[i], col 1026 = gb (F gather)
  sc  [8064, 1152]  fp16 : SC[v, j] = S[v, u_lo+j]; col 1024 = user_bias[v]/8,
                           cols 1025/1026 = 1/8 (E gather)
  uw/iw [128, 512] int16 : user/item indices (item-sorted), dma_gather layout

Per core:
  Phase A: stream r8 tiles [128, 2, 1024]; mask = sign(r8) on the Scalar
           engine (the only phase-A use of it); s += ones^T @ r8,
           c += ones^T @ mask as fp8 DoubleRow matmuls (256 rows/instr,
           the PE is phase A's pacer); ub = s/max(c,1) computed in a
           transposed [128, 8] layout (128-wide ops), then broadcast to
           ubb [128, 1024] fp16 via a PE outer product.
  Phase B: dma_gathers (b-on-partitions, [128, 4, 1152]) of E rows (by
           user) and F rows (by item) on alternating SWDGE rings;
           p1[b] = sum E*F (STT, NO ub dependency — overlaps phase A for
           every chunk gathered early; et pool depth controls how many);
           p2[b] = sum E*ubb (STT, after ub). p1 for early chunks is
           issued before any ub-dependent DVE op (in-order queue).
  Phase C: p = p1 - p2 per group of 16 output columns; AllReduce in 4
           pipelined groups; sigmoid * 5; output [128, 64] (b = j*128+p),
           un-permuted on the host.
"""

import numpy as np
import ml_dtypes
from contextlib import ExitStack

import concourse.bass as bass
import concourse.bacc as bacc
import concourse.tile as tile
from concourse import mybir
from concourse.bass_utils import run_bass_kernel_spmd

F32 = mybir.dt.float32
F16 = mybir.dt.float16
F8 = mybir.dt.float8e4
I16 = mybir.dt.int16
NPF16 = np.float16
NPF8 = ml_dtypes.float8_e4m3

NCORES = 8
U = 8001
I = 16001
B = 8192
UPC = 1024          # users per core (padded; last core has 833 real)
NU = 1024           # user columns in rt/sc
D = 1152            # gathered row width: 1024 u-cols + 3 bias cols + pad
IP = 16128          # padded item rows (63 * 256)
SCR = 8064          # sc rows (63 * 128); only rows 0..8000 are gathered
NT8 = 63            # phase-A stream tiles of 256 interleaved rows
NB_CHUNK = 512      # idxs per dma_gather call
NCHUNK = B // NB_CHUNK          # 16
SUB = NB_CHUNK // 128           # 4 sub-tiles per gather chunk
NBCOL = B // 128                # 64 columns of the [128, 64] partial/output
PD = 1027           # p1 product width (1024 u + 3 bias cols)

ET_BUFS = 11        # E-gather pool depth: chunks gatherable before ub
FT_BUFS = 3         # F tiles are freed by p1 immediately
NGROUP = 4          # AllReduce groups
GCOL = NBCOL // NGROUP          # output columns per group

_CACHED = {}


def build_program(nq=4):
    """Build the SPMD bass program (identical on all 8 cores)."""
    nc = bacc.Bacc(num_devices=NCORES, num_swdge_queues=nq)

    r8 = nc.dram_tensor("r8", [NT8 * 128, 2048], F8, kind="ExternalInput")
    rt = nc.dram_tensor("rt", [IP, D], F16, kind="ExternalInput")
    sc = nc.dram_tensor("sc", [SCR, D], F16, kind="ExternalInput")
    uw = nc.dram_tensor("uw", [128, B // 16], I16, kind="ExternalInput")
    iw = nc.dram_tensor("iw", [128, B // 16], I16, kind="ExternalInput")
    out = nc.dram_tensor("out", [128, NBCOL], F32, kind="ExternalOutput")

    with ExitStack() as ctx:
        tc = ctx.enter_context(tile.TileContext(nc))
        singles = ctx.enter_context(tc.tile_pool(name="singles", bufs=1))
        r8_pool = ctx.enter_context(tc.tile_pool(name="r8_pool", bufs=4))
        m8_pool = ctx.enter_context(tc.tile_pool(name="m8_pool", bufs=4))
        psA = ctx.enter_context(tc.tile_pool(name="psA", bufs=1, space="PSUM"))
        psB = ctx.enter_context(tc.tile_pool(name="psB", bufs=1, space="PSUM"))
        ge_pool = ctx.enter_context(tc.tile_pool(name="ge_pool", bufs=ET_BUFS))
        gf_pool = ctx.enter_context(tc.tile_pool(name="gf_pool", bufs=FT_BUFS))
        t1_pool = ctx.enter_context(tc.tile_pool(name="t1_pool", bufs=2))
        t2_pool = ctx.enter_context(tc.tile_pool(name="t2_pool", bufs=2))
        pc_pool = ctx.enter_context(tc.tile_pool(name="pc_pool", bufs=2))
        dram = ctx.enter_context(tc.tile_pool(name="dram", bufs=1, space="DRAM"))

        # fp8 DoubleRow weights: [128, 2, 1] slice of a 16-col tile (the
        # k-tile stride must be 16B-aligned for the dual-fp8 LDWEIGHTS).
        ones8 = singles.tile([128, 2, 16], F8)
        nc.vector.memset(ones8, 1.0)
        ones_row16 = singles.tile([1, 128], F16)
        nc.vector.memset(ones_row16, 1.0)

        uw_sb = singles.tile([128, B // 16], I16)
        nc.sync.dma_start(uw_sb, uw[:, :])
        iw_sb = singles.tile([128, B // 16], I16)
        nc.sync.dma_start(iw_sb, iw[:, :])

        # ---- Phase A: masked-mean reductions over the core's 1024 user cols.
        # Each stream tile holds 256 item-rows DoubleRow-interleaved:
        # [128, 2, 1024] with element [p, i, j] = R8[row 256t+128i+p, col j].
        s_ps = psA.tile([1, NU], F32)
        c_ps = psA.tile([1, NU], F32)
        for t in range(NT8):
            r8t = r8_pool.tile([128, 2, NU], F8)
            nc.sync.dma_start(
                r8t, r8[t * 128:(t + 1) * 128, :].rearrange(
                    "p (i n) -> p i n", i=2)
            )
            m8 = m8_pool.tile([128, 2, NU], F8)
            nc.scalar.activation(m8, r8t, mybir.ActivationFunctionType.Sign)
            for h in range(2):
                nc.tensor.matmul(
                    s_ps[0:1, h * 512:(h + 1) * 512],
                    ones8[:, :, 0:1],
                    r8t[:, :, h * 512:(h + 1) * 512],
                    start=(t == 0),
                    stop=(t == NT8 - 1),
                    perf_mode=mybir.MatmulPerfMode.DoubleRow,
                )
                nc.tensor.matmul(
                    c_ps[0:1, h * 512:(h + 1) * 512],
                    ones8[:, :, 0:1],
                    m8[:, :, h * 512:(h + 1) * 512],
                    start=(t == 0),
                    stop=(t == NT8 - 1),
                    perf_mode=mybir.MatmulPerfMode.DoubleRow,
                )

        # ---- Phase B part 1: gathers + p1 for the early chunks. Everything
        # here precedes the first ub-dependent DVE op in issue order, so the
        # in-order DVE queue cannot head-of-line block p1 behind p2.
        idx_w = NB_CHUNK // 16
        p1 = singles.tile([128, NBCOL], F32)
        p2 = singles.tile([128, NBCOL], F32)
        ets = {}
        fts = {}

        def issue_gather(c):
            et = ge_pool.tile([128, SUB, D], F16, name="et")
            nc.gpsimd.dma_gather(
                et, sc[:, :], uw_sb[:, c * idx_w:(c + 1) * idx_w],
                NB_CHUNK, NB_CHUNK, D,
                queue_num=(2 * (c % 2)) % nq,
            )
            ft = gf_pool.tile([128, SUB, D], F16, name="ft")
            nc.gpsimd.dma_gather(
                ft, rt[:, :], iw_sb[:, c * idx_w:(c + 1) * idx_w],
                NB_CHUNK, NB_CHUNK, D,
                queue_num=(2 * (c % 2) + 1) % nq,
            )
            ets[c] = et
            fts[c] = ft

        def issue_p1(c):
            et, ft = ets[c], fts.pop(c)
            for s in range(SUB):
                j = c * SUB + s
                t1 = t1_pool.tile([128, PD], F16)
                nc.vector.scalar_tensor_tensor(
                    out=t1, in0=ft[:, s, 0:PD], scalar=1.0,
                    in1=et[:, s, 0:PD],
                    op0=mybir.AluOpType.mult, op1=mybir.AluOpType.mult,
                    accum_out=p1[:, j:j + 1],
                )

        def issue_p2(c):
            et = ets.pop(c)
            for s in range(SUB):
                j = c * SUB + s
                t2 = t2_pool.tile([128, NU], F16)
                nc.vector.scalar_tensor_tensor(
                    out=t2, in0=et[:, s, 0:NU], scalar=1.0, in1=ubb,
                    op0=mybir.AluOpType.mult, op1=mybir.AluOpType.mult,
                    accum_out=p2[:, j:j + 1],
                )

        n_early = min(ET_BUFS, NCHUNK)
        for c in range(n_early):
            issue_gather(c)
            issue_p1(c)

        # ---- ub = s / max(c, 1) in the transposed [128, 8] layout (every
        # op 128-wide; the single-partition [1,1024] chain costs ~16 us).
        # SBUF rearrange cannot cross the partition axis, so reshapes go
        # through DRAM (linear).
        sc_sb = singles.tile([1, 2 * NU], F32)
        nc.vector.tensor_copy(sc_sb[0:1, 0:NU], s_ps[0:1, :])
        nc.vector.tensor_copy(sc_sb[0:1, NU:2 * NU], c_ps[0:1, :])
        sc_dram = dram.tile([1, 2 * NU], F32, name="sc_dram")
        nc.sync.dma_start(sc_dram, sc_sb)
        scT = singles.tile([128, 2, 8], F32)
        nc.sync.dma_start(
            scT, sc_dram[0:1, :].rearrange("o (a k p) -> (o p) a k", a=2, k=8)
        )
        cmaxT = singles.tile([128, 8], F32)
        nc.vector.tensor_scalar_max(cmaxT, scT[:, 1, :], 1.0)
        crecT = singles.tile([128, 8], F32)
        nc.vector.reciprocal(crecT, cmaxT)
        ubT16 = singles.tile([128, 8], F16)
        nc.vector.tensor_tensor(
            ubT16, scT[:, 0, :], crecT, mybir.AluOpType.mult
        )
        # back to row layout [1, 1024] fp16 (write side rearranged; DRAM
        # is linear so this is legal), then broadcast across partitions
        # with a PE outer product.
        ub_dram = dram.tile([1, NU], F16, name="ub_dram")
        nc.sync.dma_start(
            ub_dram[0:1, :].rearrange("o (k p) -> (o p) k", k=8), ubT16
        )
        ub_row = singles.tile([1, NU], F16)
        nc.sync.dma_start(ub_row, ub_dram[0:1, :])
        ubb_ps = psB.tile([128, NU], F32)
        for h in range(2):
            nc.tensor.matmul(
                ubb_ps[:, h * 512:(h + 1) * 512],
                ones_row16[:, :],
                ub_row[0:1, h * 512:(h + 1) * 512],
                start=True,
                stop=True,
            )
        ubb = singles.tile([128, NU], F16)
        nc.vector.tensor_copy(ubb, ubb_ps[:, :])

        # ---- Phase B part 2: interleave p2 (freeing et bufs) with the
        # remaining gathers + p1.
        for c in range(n_early, NCHUNK):
            issue_p2(c - n_early)
            issue_gather(c)
            issue_p1(c)
        for c in range(NCHUNK - n_early, NCHUNK):
            issue_p2(c)

        # ---- Phase C: p = p1 - p2 per group; pipelined AllReduces.
        for g in range(NGROUP):
            lo, hi = g * GCOL, (g + 1) * GCOL
            pg = pc_pool.tile([128, GCOL], F32, name="pg")
            nc.vector.tensor_tensor(
                pg, p1[:, lo:hi], p2[:, lo:hi], mybir.AluOpType.subtract
            )
            cc_in = dram.tile([128, GCOL], F32, name=f"cci{g}")
            cc_out = dram.tile([128, GCOL], F32, name=f"cco{g}")
            nc.sync.dma_start(cc_in, pg)
            nc.gpsimd.collective_compute(
                "AllReduce",
                mybir.AluOpType.add,
                replica_groups=[list(range(NCORES))],
                ins=[cc_in.opt()],
                outs=[cc_out.opt()],
            )
            pred = pc_pool.tile([128, GCOL], F32, name="pred")
            nc.sync.dma_start(pred, cc_out)
            nc.scalar.activation(
                pred, pred, mybir.ActivationFunctionType.Sigmoid
            )
            nc.scalar.mul(pred, pred, 5.0)
            nc.sync.dma_start(out[:, lo:hi], pred)

    nc.finalize()
    return nc


def _wrap_idxs(ix: np.ndarray) -> np.ndarray:
    """dma_gather wrapped layout: idx i of the list lives at [i % 16, i // 16],
    replicated across the eight 16-partition groups."""
    a = np.ascontiguousarray(ix.astype(np.int16).reshape(B // 16, 16).T)
    return np.ascontiguousarray(np.tile(a, (8, 1)))


def prepare_inputs(user, item, rating_mtx, user_similarity, user_bias,
                   item_bias, global_bias):
    user = np.asarray(user).astype(np.int64)
    item = np.asarray(item).astype(np.int64)
    R = np.asarray(rating_mtx, dtype=np.float32)
    S = np.asarray(user_similarity, dtype=np.float32)
    ubias = np.asarray(user_bias, dtype=np.float32)
    ibias = np.asarray(item_bias, dtype=np.float32)
    gb = np.float32(np.asarray(global_bias))

    perm = np.argsort(item, kind="stable")
    item_s = item[perm]
    user_s = user[perm]

    uw = _wrap_idxs(user_s)
    iw = _wrap_idxs(item_s)

    in_maps = []
    for k in range(NCORES):
        u_lo = k * UPC
        u_hi = min(u_lo + UPC, U)
        nu = u_hi - u_lo

        rt = np.zeros((IP, D), NPF16)
        rt[:I, :nu] = R[u_lo:u_hi, :].T.astype(NPF16)
        rt[:I, 1024] = NPF16(1.0)
        rt[:I, 1025] = ibias.astype(NPF16)
        rt[:I, 1026] = NPF16(gb)

        # phase-A fp8 stream, DoubleRow interleaved: DRAM row 128t+p holds
        # item-rows 256t+p and 256t+128+p of the transposed rating slice.
        r8full = np.zeros((IP, NU), NPF8)
        r8full[:I, :nu] = R[u_lo:u_hi, :].T.astype(NPF8)
        r8 = np.ascontiguousarray(
            r8full.reshape(NT8, 2, 128, NU).transpose(0, 2, 1, 3)
            .reshape(NT8 * 128, 2 * NU)
        )

        sc = np.zeros((SCR, D), NPF16)
        sc[:U, :nu] = S[:, u_lo:u_hi].astype(NPF16)
        sc[:U, 1024] = (ubias / np.float32(NCORES)).astype(NPF16)
        sc[:U, 1025] = NPF16(1.0 / NCORES)
        sc[:U, 1026] = NPF16(1.0 / NCORES)

        in_maps.append({"r8": r8, "rt": rt, "sc": sc, "uw": uw, "iw": iw})
    return in_maps, perm


def kernel(user, item, rating_mtx, user_similarity, user_bias, item_bias,
           global_bias, _trace=False):
    if "nc" not in _CACHED:
        _CACHED["nc"] = build_program()
    nc = _CACHED["nc"]

    in_maps, perm = prepare_inputs(
        user, item, rating_mtx, user_similarity, user_bias, item_bias,
        global_bias,
    )
    res = run_bass_kernel_spmd(nc, in_maps, core_ids=list(range(NCORES)))
    if _trace:
        # cold traced runs have hung; trace only after a warm run
        res = run_bass_kernel_spmd(
            nc, in_maps, core_ids=list(range(NCORES)), trace=True
        )
    _CACHED["last_results"] = res

    o = np.asarray(res.results[0]["out"])          # [128, 64], b = j*128 + p
    p_sorted = np.ascontiguousarray(o.T).reshape(-1)
    out = np.empty(B, np.float32)
    out[perm] = p_sorted
    return out
